# revision 2
# baseline (speedup 1.0000x reference)
"""Trainium2 Bass kernel v3 for nn_CrystalNet (dense transformer, 8 cores).

Design vs v2 baseline:
- Head-sharded attention: one AllToAll redistributes k/v from token-sharding
  (core c owns batch0 block c + batch1 block 7-c) to head-sharding (core c
  owns head c, all 2048 tokens).  Causality is then exploited EXACTLY and
  uniformly (same instruction stream on every core): per batch, j-block jb
  only attends queries q >= 128*jb, so energy/exp/pv/denominator work is
  9/16 of the all-blocks baseline, with wide (<=512) free dims.
- Batch-1 queries are stored in reversed block order so its causal structure
  becomes a prefix pattern identical in shape to batch 0.
- Masking is a single [128,128] tril multiply on diagonal tiles only.
- Second AllToAll (per batch, staggered for overlap) returns pairwise values
  to token-sharding for the Wo matmul.
- fp8 everywhere on the PE: weights prescaled by 64 on host, descaled during
  PSUM evacuation.  v-projection and the LM head use fp8 DoubleRow (2
  contraction tiles per instruction).
- so = h@Wso.T is added into the force PSUM banks via an identity matmul
  instead of a separate DVE pass.
- Softmax denominators: ones-matmul column sums -> reciprocal_approx_fast on
  one partition -> ones-broadcast matmul -> fused multiply at pv evacuation.
"""
import math
import sys

import numpy as np

sys.path.insert(0, "/opt/trn_rl_repo")

B, S, D, H, IT, V = 2, 1024, 1024, 8, 28, 50257
P = 128
DH = D // H          # 128
NDT = D // P         # 8
NC = 8
Q = 128
TPC = B * Q          # 256 tokens per core
VS = 6284            # padded vocab shard
NVB = (VS + 511) // 512
EPS = 1.1920929e-07
ATT_SCALE = -1.0 / math.sqrt(DH)
WS = 64.0            # weight prescale for fp8

_CACHE = {}


def _build(step_size, its=IT, with_lm=True, dump=None):
    import concourse.bacc as bacc
    import concourse.tile as tile
    import concourse.mybir as mybir

    f32 = mybir.dt.float32
    bf16 = mybir.dt.bfloat16
    f8 = mybir.dt.float8e4
    AF = mybir.ActivationFunctionType
    ALU = mybir.AluOpType
    DR = mybir.MatmulPerfMode.DoubleRow

    nc = bacc.Bacc("TRN2", target_bir_lowering=False, debug=False, num_devices=NC)

    # ---- DRAM I/O (per core) ----
    x0T_in = nc.dram_tensor("x0T", [P, NDT, TPC], f32, kind="ExternalInput")
    wk_in = nc.dram_tensor("wkT", [NDT, P, D], bf16, kind="ExternalInput")
    wv_in = nc.dram_tensor("wvT", [NDT, P, D], bf16, kind="ExternalInput")
    wso_in = nc.dram_tensor("wsoT", [NDT, P, D], bf16, kind="ExternalInput")
    wo_in = nc.dram_tensor("woT", [H, P, D], bf16, kind="ExternalInput")
    nwg_in = nc.dram_tensor("nwgc", [P, NDT, IT], f32, kind="ExternalInput")
    onw_in = nc.dram_tensor("onwc", [P, NDT, 1], f32, kind="ExternalInput")
    tril_in = nc.dram_tensor("trilc", [P, Q], bf16, kind="ExternalInput")
    ident_in = nc.dram_tensor("identc", [P, P], bf16, kind="ExternalInput")
    onec_in = nc.dram_tensor("onec", [P, 1], bf16, kind="ExternalInput")
    lm_in = nc.dram_tensor("lmT", [NDT, P, VS], bf16, kind="ExternalInput")
    out_ext = nc.dram_tensor("out", [2 * NC * P, VS], bf16, kind="ExternalOutput")
    dmp_ext = None
    if dump is not None:
        DSHAPES = dict(x=([P, NDT, TPC], f32), hq=([P, NDT, TPC], f32),
                       kq=([P, H, TPC], f32), vq=([P, B, D], f32),
                       Kall=([P, 16, P], f32), Vall=([P, 16, P], f32),
                       et0=([P, 8, 512], f32), pw0=([P, 8, P], f32),
                       pw1=([P, 8, P], f32),
                       PW0=([P, H, P], f32), dh0=([1, 512], f32),
                       so=([P, NDT, TPC], f32))
        dshape, ddt = DSHAPES[dump]
        dmp_ext = nc.dram_tensor("dmp", dshape, ddt, kind="ExternalOutput")

    rg = [list(range(NC))]

    with tile.TileContext(nc) as tc:
        with (
            tc.tile_pool(name="res", bufs=1) as res,
            tc.tile_pool(name="ps", bufs=1, space="PSUM") as ps,
            tc.tile_pool(name="dram", bufs=2, space="DRAM") as dram,
        ):
            # ---- residents ----
            wk_sb = res.tile([P, NDT, D], bf16)
            wv_sb = res.tile([P, NDT, D], bf16)
            wso_sb = res.tile([P, NDT, D], bf16)
            for w_sb, w_in in ((wk_sb, wk_in), (wv_sb, wv_in), (wso_sb, wso_in)):
                nc.sync.dma_start(out=w_sb[:], in_=w_in.ap().rearrange("a p f -> p a f"))
            wo_sb = res.tile([P, H, D], bf16)
            nc.sync.dma_start(out=wo_sb[:], in_=wo_in.ap().rearrange("h p f -> p h f"))
            x_sb = res.tile([P, NDT, TPC], f32)
            nc.sync.dma_start(out=x_sb[:], in_=x0T_in[:])
            nwgc_sb = res.tile([P, NDT, IT], f32)
            nc.sync.dma_start(out=nwgc_sb[:], in_=nwg_in[:])
            onwc_sb = res.tile([P, NDT, 1], f32)
            nc.sync.dma_start(out=onwc_sb[:], in_=onw_in[:])
            tril_sb = res.tile([P, Q], bf16)
            nc.sync.dma_start(out=tril_sb[:], in_=tril_in[:])
            ident_sb = res.tile([P, P], bf16)
            nc.sync.dma_start(out=ident_sb[:], in_=ident_in[:])
            onec_sb = res.tile([P, 1], bf16)
            nc.sync.dma_start(out=onec_sb[:], in_=onec_in[:])
            oner_bf = res.tile([1, P], bf16)
            nc.vector.memset(oner_bf[:], 1.0)
            epsc = res.tile([1, 1], f32)
            nc.vector.memset(epsc[:], EPS)

            agy_out = dram.tile([NC, P, NDT, TPC], bf16, tag="agyo", bufs=1,
                                addr_space="Shared", name="agy_out")

            with tc.tile_pool(name="stg", bufs=2) as stg:

                def norm_quant(col_sb, it_or_none, dst_sb, dst_dt):
                    """dst = (x * rsqrt(mean(x^2)+eps) * col) quantized to dst_dt."""
                    sq = stg.tile([P, NDT, TPC], bf16, tag="sqz", bufs=1)
                    nc.vector.tensor_mul(sq[:], x_sb[:], x_sb[:])
                    ssq = ps.tile([P, 512], f32, tag="small", bufs=2)
                    for dt in range(NDT):
                        nc.tensor.matmul(ssq[0:1, 0:TPC], onec_sb[:], sq[:, dt, :],
                                         start=(dt == 0), stop=(dt == NDT - 1))
                    lnv = stg.tile([1, TPC], f32, tag="lnv")
                    nc.scalar.activation(lnv[:], ssq[0:1, 0:TPC], AF.Ln,
                                         scale=1.0 / D, bias=epsc[:])
                    rstd = stg.tile([1, TPC], bf16, tag="rstd")
                    nc.scalar.activation(rstd[:], lnv[:], AF.Exp, scale=-0.5)
                    rb = ps.tile([P, 512], f32, tag="small", bufs=2)
                    nc.tensor.matmul(rb[:, 0:TPC], oner_bf[:], rstd[:],
                                     start=True, stop=True)
                    for dt in range(NDT):
                        sc_ap = (col_sb[:, dt, it_or_none:it_or_none + 1]
                                 if it_or_none is not None else col_sb[:, dt, 0:1])
                        nc.vector.scalar_tensor_tensor(
                            dst_sb[:, dt, :], x_sb[:, dt, :], sc_ap, rb[:, 0:TPC],
                            ALU.mult, ALU.mult)

                for it in range(its):
                    st64 = float(step_size[it % IT])

                    # ---- norm (bf16 out) ----
                    hq = stg.tile([P, NDT, TPC], bf16, tag="hq", bufs=2)
                    norm_quant(nwgc_sb, it % IT, hq, bf16)

                    # ---- k projection: kq [dh, h, t] (feature-major) ----
                    kq = stg.tile([P, H, TPC], f8, tag="kq", bufs=2)
                    for h in range(H):
                        kp = ps.tile([P, 512], f32, tag="ep", bufs=2)
                        for dt in range(NDT):
                            nc.tensor.matmul(kp[:, 0:TPC],
                                             wk_sb[:, dt, h * P:(h + 1) * P],
                                             hq[:, dt, :],
                                             start=(dt == 0), stop=(dt == NDT - 1))
                        if h % 2:
                            nc.vector.tensor_copy(kq[:, h, :], kp[:, 0:TPC])
                        else:
                            nc.scalar.copy(kq[:, h, :], kp[:, 0:TPC])

                    # ---- v projection (token-major, bf16) ----
                    vq = stg.tile([P, B, D], f8, tag="vq", bufs=2)
                    for tb in range(B):
                        for fb in range(2):
                            vp = ps.tile([P, 512], f32, tag="pv", bufs=2)
                            for dt in range(NDT):
                                nc.tensor.matmul(
                                    vp[:],
                                    hq[:, dt, tb * P:(tb + 1) * P],
                                    wv_sb[:, dt, fb * 512:(fb + 1) * 512],
                                    start=(dt == 0), stop=(dt == NDT - 1))
                            if fb:
                                nc.vector.tensor_copy(
                                    vq[:, tb, fb * 512:(fb + 1) * 512], vp[:])
                            else:
                                nc.scalar.copy(
                                    vq[:, tb, fb * 512:(fb + 1) * 512], vp[:])

                    # ---- AllToAll #1: redistribute k/v to head-sharding ----
                    agi = dram.tile([NC, P, 4 * P], f8, tag="agi")
                    ago = dram.tile([NC, P, 4 * P], f8, tag="ago")
                    nc.sync.dma_start(out=agi[:, :, 0:TPC].rearrange("h p t -> p h t"),
                                      in_=kq[:])
                    nc.sync.dma_start(
                        out=agi[:, :, TPC:TPC + P].rearrange("h p t -> p h t"),
                        in_=vq[:, 0, :].rearrange("p (h d) -> p h d", h=H))
                    nc.sync.dma_start(
                        out=agi[:, :, TPC + P:].rearrange("h p t -> p h t"),
                        in_=vq[:, 1, :].rearrange("p (h d) -> p h d", h=H))
                    nc.gpsimd.collective_compute("AllToAll", ALU.bypass,
                                                 replica_groups=rg,
                                                 ins=[agi.opt()], outs=[ago.opt()])

                    # ---- so = h @ Wso.T while the AllToAll is in flight ----
                    so_sb = stg.tile([P, NDT, TPC], bf16, tag="sos", bufs=2)
                    for s_ in range(NDT):
                        sp = ps.tile([P, 512], f32, tag="fo", bufs=2)
                        for dt in range(NDT):
                            nc.tensor.matmul(sp[:, 0:TPC],
                                             wso_sb[:, dt, s_ * P:(s_ + 1) * P],
                                             hq[:, dt, :],
                                             start=(dt == 0), stop=(dt == NDT - 1))
                        if s_ % 2:
                            nc.vector.tensor_copy(so_sb[:, s_, :], sp[:, 0:TPC])
                        else:
                            nc.scalar.copy(so_sb[:, s_, :], sp[:, 0:TPC])

                    # ---- load gathered K/V: index j<8 -> batch0 block j;
                    #      index 8+p -> batch1 block 7-p (reversed q order) ----
                    K_all = stg.tile([P, 16, P], f8, tag="Kall", bufs=2)
                    V_all = stg.tile([P, 16, P], f8, tag="Vall", bufs=2)
                    nc.sync.dma_start(out=K_all[:, 0:8, :],
                                      in_=ago[:, :, 0:P].rearrange("r p t -> p r t"))
                    nc.sync.dma_start(out=K_all[:, 8:16, :],
                                      in_=ago[:, :, P:TPC].rearrange("r p t -> p r t"))
                    nc.sync.dma_start(out=V_all[:, 0:8, :],
                                      in_=ago[:, :, TPC:TPC + P]
                                      .rearrange("r p t -> p r t"))
                    nc.sync.dma_start(out=V_all[:, 8:16, :],
                                      in_=ago[:, :, TPC + P:]
                                      .rearrange("r p t -> p r t"))

                    if it == 0 and dump in ("hq", "kq", "vq", "Kall", "Vall", "so"):
                        src = dict(hq=hq, kq=kq, vq=vq, Kall=K_all, Vall=V_all,
                                   so=so_sb)[dump]
                        dcp = stg.tile(list(src.shape), f32, tag="dcp", bufs=1,
                                       name="dcp")
                        nc.vector.tensor_copy(dcp[:], src[:])
                        nc.sync.dma_start(out=dmp_ext[:], in_=dcp[:])

                    # ---- attention per batch (head c implicit via data) ----
                    pw = [None, None]
                    ago2 = [None, None]
                    for b in range(B):
                        joff = 8 * b
                        pw_b = stg.tile([P, 8, P], bf16, tag=f"pw{b}", bufs=2)
                        for cch in range(2):
                            # j-blocks needed for this 512-wide q chunk
                            if b == 0:
                                jlist = list(range(0, 4 * cch + 4))
                            else:
                                jlist = list(range(0, 8 if cch == 0 else 4))
                            et = stg.tile([P, 8, 512], bf16, tag="et", bufs=2)
                            pvp = ps.tile([P, 512], f32, tag="pv", bufs=2)
                            dhp = ps.tile([P, 512], f32, tag="small", bufs=2)
                            for ji, jb in enumerate(jlist):
                                if b == 0:
                                    jidx = jb
                                    qs = max(512 * cch, 128 * jb) - 512 * cch
                                    qe = 512
                                    qb0 = (512 * cch + qs * 1) // 128
                                    rhs = K_all[:, qb0:4 * cch + 4, :]
                                else:
                                    jidx = 8 + 7 - jb
                                    qs = 0
                                    qe = min(512, 128 * (8 - jb) - 512 * cch)
                                    qb0 = 8 + 4 * cch
                                    rhs = K_all[:, qb0:qb0 + (qe // 128), :]
                                n = qe - qs
                                ep = ps.tile([P, 512], f32, tag="ep", bufs=2)
                                nc.tensor.matmul(ep[:, 0:n], K_all[:, jidx, :], rhs,
                                                 start=True, stop=True)
                                nc.scalar.activation(et[:, ji, qs:qe], ep[:, 0:n],
                                                     AF.Exp, scale=ATT_SCALE)
                                # diagonal tril mask
                                if b == 0:
                                    dq = jb - 4 * cch
                                else:
                                    dq = (7 - jb) - 4 * cch
                                if 0 <= dq < 4:
                                    dcol = 128 * dq
                                    nc.vector.tensor_mul(
                                        et[:, ji, dcol:dcol + 128],
                                        et[:, ji, dcol:dcol + 128], tril_sb[:])
                                last = (ji == len(jlist) - 1)
                                nc.tensor.matmul(dhp[0:1, qs:qe], onec_sb[:],
                                                 et[:, ji, qs:qe],
                                                 start=(ji == 0), stop=last)
                                nc.tensor.matmul(pvp[:, qs:qe],
                                                 V_all[:, jidx, :],
                                                 et[:, ji, qs:qe],
                                                 start=(ji == 0), stop=last)
                            if it == 0 and dump == "et0" and b == 0 and cch == 1:
                                dcp = stg.tile([P, 8, 512], f32, tag="dcp",
                                               bufs=1, name="dcp2")
                                nc.vector.tensor_copy(dcp[:], et[:])
                                nc.sync.dma_start(out=dmp_ext[:], in_=dcp[:])
                            # normalize: pw = pv / denom (fp8 out)
                            dh_sb = stg.tile([1, 512], f32, tag="dhs")
                            nc.vector.reciprocal_approx_fast(dh_sb[:],
                                                             dhp[0:1, :])
                            dh_bf = stg.tile([1, 512], bf16, tag="dhb")
                            nc.scalar.copy(dh_bf[:], dh_sb[:])
                            rcb = ps.tile([P, 512], f32, tag="small", bufs=2)
                            nc.tensor.matmul(rcb[:], oner_bf[:], dh_bf[:],
                                             start=True, stop=True)
                            rs = stg.tile([P, 512], bf16, tag="rs", bufs=2)
                            nc.vector.tensor_copy(rs[:], rcb[:])
                            nc.vector.tensor_mul(
                                pw_b[:].rearrange("p a t -> p (a t)")
                                [:, 512 * cch:512 * cch + 512],
                                pvp[:], rs[:])
                        if it == 0 and dump == f"pw{b}":
                            dcp = stg.tile([P, 8, P], f32, tag="dcp", bufs=1,
                                           name="dcp3")
                            nc.vector.tensor_copy(dcp[:], pw_b[:])
                            nc.sync.dma_start(out=dmp_ext[:], in_=dcp[:])
                        # ---- AllToAll #2 for this batch ----
                        agi2 = dram.tile([NC, P, P], bf16, tag=f"agi2{b}",
                                         name=f"agi2{b}")
                        ago2[b] = dram.tile([NC, P, P], bf16, tag=f"ago2{b}",
                                            name=f"ago2{b}")
                        nc.sync.dma_start(out=agi2[:].rearrange("r p t -> p r t"),
                                          in_=pw_b[:])
                        nc.gpsimd.collective_compute("AllToAll", ALU.bypass,
                                                     replica_groups=rg,
                                                     ins=[agi2.opt()],
                                                     outs=[ago2[b].opt()])
                        pw[b] = pw_b

                    # ---- force = pw @ Wo.T + so; x += st * force ----
                    for b in range(B):
                        PW = stg.tile([P, H, P], bf16, tag=f"PW{b}", bufs=2)
                        nc.sync.dma_start(out=PW[:],
                                          in_=ago2[b][:].rearrange("h p t -> p h t"))
                        for s_ in range(NDT):
                            fo = ps.tile([P, 512], f32, tag="fo", bufs=2)
                            for h in range(H):
                                nc.tensor.matmul(fo[:, 0:P],
                                                 wo_sb[:, h, s_ * P:(s_ + 1) * P],
                                                 PW[:, h, :],
                                                 start=(h == 0), stop=False)
                            nc.tensor.matmul(fo[:, 0:P], ident_sb[:],
                                             so_sb[:, s_, b * P:(b + 1) * P],
                                             start=False, stop=True)
                            nc.vector.scalar_tensor_tensor(
                                x_sb[:, s_, b * P:(b + 1) * P], fo[:, 0:P], st64,
                                x_sb[:, s_, b * P:(b + 1) * P], ALU.mult, ALU.add)

                if dump == "x":
                    dcp = stg.tile([P, NDT, TPC], f32, tag="dcp", bufs=1,
                                   name="dcp4")
                    nc.vector.tensor_copy(dcp[:], x_sb[:])
                    nc.sync.dma_start(out=dmp_ext[:], in_=dcp[:])
                # ---- final norm + AllGather of y ----
                yq = stg.tile([P, NDT, TPC], bf16, tag="hq", bufs=2)
                norm_quant(onwc_sb, None, yq, bf16)
                agy_in = dram.tile([P, NDT, TPC], bf16, tag="agyi", bufs=1)
                nc.sync.dma_start(out=agy_in[:], in_=yq[:])
                nc.gpsimd.collective_compute("AllGather", ALU.bypass,
                                             replica_groups=rg,
                                             ins=[agy_in.opt()],
                                             outs=[agy_out.opt()])

            # ---- LM head: fp8 DoubleRow over vocab shard ----
            with tc.tile_pool(name="lmp", bufs=2) as lmp:
                y_all = lmp.tile([P, NDT, NC * TPC], bf16, tag="yall", bufs=1)
                for r in range(NC):
                    nc.sync.dma_start(out=y_all[:, :, r * TPC:(r + 1) * TPC],
                                      in_=agy_out[r])
                for vb in range(NVB):
                    nv = min(512, VS - vb * 512)
                    lw = lmp.tile([P, NDT, 512], bf16, tag="lw", bufs=3)
                    nc.sync.dma_start(
                        out=lw[:, :, :nv],
                        in_=lm_in.ap()[:, :, vb * 512:vb * 512 + nv]
                            .rearrange("a p v -> p a v"))
                    for ts in range(2 * NC):
                        op = ps.tile([P, 512], f32, tag="ep", bufs=2)
                        for dt in range(NDT):
                            nc.tensor.matmul(
                                op[:, :nv],
                                y_all[:, dt, ts * P:(ts + 1) * P],
                                lw[:, dt, :nv],
                                start=(dt == 0), stop=(dt == NDT - 1))
                        ob = lmp.tile([P, 512], bf16, tag="ob", bufs=4)
                        if ts % 2:
                            nc.vector.tensor_copy(ob[:, :nv], op[:, :nv])
                        else:
                            nc.scalar.copy(ob[:, :nv], op[:, :nv])
                        nc.sync.dma_start(
                            out=out_ext[ts * P:(ts + 1) * P,
                                        vb * 512:vb * 512 + nv],
                            in_=ob[:, :nv])
    nc.compile()
    return nc


def _make_runner(nc, n_cores):
    import jax
    import jax.numpy as jnp
    from jax.sharding import Mesh, PartitionSpec, NamedSharding
    from jax.experimental.shard_map import shard_map
    import concourse.mybir as mybir
    from concourse.bass2jax import (_bass_exec_p, install_neuronx_cc_hook,
                                    partition_id_tensor)

    install_neuronx_cc_hook()
    partition_name = nc.partition_id_tensor.name if nc.partition_id_tensor else None
    in_names, out_names, out_avals = [], [], []
    for alloc in nc.m.functions[0].allocations:
        if not isinstance(alloc, mybir.MemoryLocationSet):
            continue
        name = alloc.memorylocations[0].name
        if alloc.kind == "ExternalInput":
            if name != partition_name:
                in_names.append(name)
        elif alloc.kind == "ExternalOutput":
            out_names.append(name)
            out_avals.append(jax.core.ShapedArray(tuple(alloc.tensor_shape),
                                                  mybir.dt.np(alloc.dtype)))
    n_params = len(in_names)
    all_in = list(in_names) + list(out_names)
    if partition_name is not None:
        all_in.append(partition_name)
    donate = tuple(range(n_params, n_params + len(out_names)))

    def _body(*args):
        operands = list(args)
        if partition_name is not None:
            operands.append(partition_id_tensor())
        return tuple(_bass_exec_p.bind(
            *operands, out_avals=tuple(out_avals), in_names=tuple(all_in),
            out_names=tuple(out_names), lowering_input_output_aliases=(),
            sim_require_finite=True, sim_require_nnan=True, nc=nc))

    devices = jax.devices()[:n_cores]
    mesh = Mesh(np.asarray(devices), ("core",))
    spec = PartitionSpec("core")
    sharding = NamedSharding(mesh, spec)
    n_out = len(out_names)
    sharded = jax.jit(
        shard_map(_body, mesh=mesh, in_specs=(spec,) * (n_params + n_out),
                  out_specs=(spec,) * n_out, check_rep=False),
        donate_argnums=donate, keep_unused=True)

    zero_fns = [
        jax.jit(lambda av=av: jnp.zeros((n_cores * av.shape[0], *av.shape[1:]),
                                        av.dtype),
                out_shardings=sharding)
        for av in out_avals
    ]

    def run(in_maps):
        per_core = [[np.asarray(m[name]) for name in in_names] for m in in_maps]
        concat_in = [np.concatenate([per_core[c][i] for c in range(n_cores)], axis=0)
                     for i in range(n_params)]
        zeros = [zf() for zf in zero_fns]
        outs = sharded(*concat_in, *zeros)
        jax.block_until_ready(outs)
        return [
            {name: np.asarray(outs[i]).reshape(n_cores, *out_avals[i].shape)[c]
             for i, name in enumerate(out_names)}
            for c in range(n_cores)
        ]
    run.in_names = in_names
    run._sharded = sharded
    run._zero_fns = zero_fns
    return run


def _f8(x):
    import ml_dtypes
    x = np.clip(np.ascontiguousarray(x, dtype=np.float32), -240.0, 240.0)
    return x.astype(ml_dtypes.float8_e4m3)


def _bf16(x):
    import ml_dtypes
    x = np.ascontiguousarray(x, dtype=np.float32)
    u = x.view(np.uint32)
    r = ((u >> 16) & 1).astype(np.uint32)
    out = ((u + 0x7FFF + r) >> 16).astype(np.uint16)
    return np.asarray(out.view(ml_dtypes.bfloat16))


def _prep_inputs(inputs):
    tokens = np.asarray(inputs["tokens"])
    embed = np.asarray(inputs["embed"], dtype=np.float32)
    step_size = np.asarray(inputs["step_size"], dtype=np.float32)
    norm_w = np.asarray(inputs["norm_w"], dtype=np.float32)
    gamma = np.asarray(inputs["gamma"], dtype=np.float32)
    beta = np.asarray(inputs["beta"], dtype=np.float32)
    out_norm_w = np.asarray(inputs["out_norm_w"], dtype=np.float32)
    lm_head = np.asarray(inputs["lm_head_w"], dtype=np.float32)
    Ws = np.asarray(inputs["Ws"], dtype=np.float32)
    Wo = np.asarray(inputs["Wo"], dtype=np.float32)
    assert not np.any(beta), "beta != 0 not supported by v3 kernel"

    x0 = embed[tokens]                      # [B, S, D] fp32
    Wso = Wo @ Ws
    Wk = np.asarray(inputs["Wk"], np.float32)
    Wv = np.asarray(inputs["Wv"], np.float32)

    wkT = _bf16(Wk.T.reshape(NDT, P, D))
    wvT = _bf16(Wv.T.reshape(NDT, P, D))
    wsoT = _bf16(Wso.T.reshape(NDT, P, D))
    woT = _bf16(Wo.T.reshape(H, P, D))

    nwgc = np.ascontiguousarray(np.transpose(
        (norm_w * gamma).reshape(IT, NDT, P), (2, 1, 0)), dtype=np.float32)
    onwc = np.ascontiguousarray(
        out_norm_w.reshape(NDT, P).T[:, :, None], dtype=np.float32)
    tril = _bf16(np.triu(np.ones((Q, Q), np.float32)))  # tril[j,q]=1 iff j<=q
    ident = _bf16(np.eye(P, dtype=np.float32))
    onec = _bf16(np.ones((P, 1), np.float32))

    lm_pad = np.zeros((NC * VS, D), np.float32)
    lm_pad[:V] = lm_head
    lmT_shards = [
        _bf16(lm_pad[c * VS:(c + 1) * VS].T.reshape(NDT, P, VS))
        for c in range(NC)
    ]

    in_maps = []
    for c in range(NC):
        blk = [c, NC - 1 - c]
        xc = np.concatenate([x0[b, blk[b] * Q:(blk[b] + 1) * Q] for b in range(B)],
                            axis=0)
        x0T = np.ascontiguousarray(
            np.transpose(xc.reshape(TPC, NDT, P), (2, 1, 0)), dtype=np.float32)
        in_maps.append(dict(
            x0T=x0T, wkT=wkT, wvT=wvT, wsoT=wsoT, woT=woT,
            nwgc=nwgc, onwc=onwc, trilc=tril, identc=ident, onec=onec,
            lmT=lmT_shards[c],
        ))
    return in_maps, step_size


def kernel_debug(dump, its, **inputs):
    in_maps, step_size = _prep_inputs(inputs)
    key = ("v3dbg", its, dump)
    if key not in _CACHE:
        nc = _build(step_size, its=its, dump=dump)
        _CACHE[key] = (nc, _make_runner(nc, NC))
    nc, run = _CACHE[key]
    return run(in_maps)


def kernel(**inputs):
    in_maps, step_size = _prep_inputs(inputs)
    key = ("v3", IT, True)
    if key not in _CACHE:
        nc = _build(step_size, its=IT, with_lm=True)
        _CACHE[key] = (nc, _make_runner(nc, NC))
    nc, run = _CACHE[key]
    results = run(in_maps)

    logits = np.empty((B, S, V), np.float32)
    for c in range(NC):
        vlo = c * VS
        take = min(VS, V - vlo) if vlo < V else 0
        if take <= 0:
            continue
        o = np.asarray(results[c]["out"]).astype(np.float32)
        for m in range(2 * NC):
            r, tc = m // 2, m % 2
            blk = r if tc == 0 else NC - 1 - r
            logits[tc, blk * Q:(blk + 1) * Q, vlo:vlo + take] = \
                o[m * P:(m + 1) * P, :take]
    return logits
